# revision 12
# baseline (speedup 1.0000x reference)
"""Trainium2 Bass kernel for nn_AttentionCl (dense transformer attention block).

Problem (hardcoded): B=8, H=W=32 (N=1024 tokens), C=512, 16 heads x dh=32.
    qkv = x @ W_qkv + b_qkv ; per (b,h): S = q k^T * dh^-0.5 + rel_pos[h]
    P = softmax(S, axis=-1) ; O = P v ; out = concat(O) @ W_proj + b_proj
Sharding: 8 cores = 2 batch-groups x 4 head-groups. Each core handles
4 batches x 4 heads and emits a partial projection output (its 4 heads'
contribution, bf16); the host sums the 4 head-group partials and adds
b_proj.

Design (all-bf16 matmul path; ACT-engine exp over 16.8M scores is the
roofline at 128 lanes x 1.2 GHz):
  - x^T, W_qkv, W_v, W_proj in bf16 (measured end-to-end rel err ~3e-3,
    budget 2e-2). q^T/k^T via W-stationary matmuls; v natural [n,d] via
    x^T-stationary matmuls.
  - S^T tiles from K=32 matmuls at per-head row strips (tile_position)
    so up to 4 run concurrently in the PE array.
  - Scores stream through a 2x3-bank PSUM rotation; ONE exp ACTIVATE per
    3-bank tile (1536 elems/lane) amortizes the ACT instruction
    overhead; output goes straight to SBUF bf16.
  - P = exp(S^T) * exp(bias^T) in bf16 (2x DVE mode) over 6-slot chunks,
    all on the vector engine (gpsimd is ~5x slower per element).
  - p_sb is split per-mb (two 32-slot buffers) and qkt is double-
    buffered so batch b+1's qk matmuls and exps never WAR-stall on
    batch b's PV reads / score reads.
  - PV: v augmented with a ones column + zero padding to 64 lanes
    (row 32/96 = softmax denoms; all PSUM rows written), 2 heads per
    PSUM bank via tile_position (0,0)/(0,64).
  - 1/denominator: one full-tile reciprocal_approx_fast off PSUM; the
    two denom rows hop to partition 0 via tiny SBUF->SBUF DMAs, then
    gpsimd partition_broadcast fans them across the head strips (the
    ucode only reads physical partition 0).
  - Projection without repacking: per-pair zero-padded W_proj tiles so
    the pair O tiles serve as K=128 lhsT directly; 2 accumulating
    matmuls per token chunk. Proj chunks 0-3 drain right after mb0's
    division so the batch tail is half as deep.
  - Emission is software-pipelined through a work queue: PV passes and
    proj/store are drained between score matmuls so neither the PE nor
    the scalar engine starves.
"""

import os
import numpy as np
import ml_dtypes

import concourse.bass as bass  # noqa: F401  (AP construction if needed)
import concourse.tile as tile
from concourse import bacc, mybir
from concourse.bass_utils import run_bass_kernel_spmd

F32 = mybir.dt.float32
BF16 = mybir.dt.bfloat16

B, N, C = 8, 1024, 512
NH, DH = 16, 32
SCALE = DH ** -0.5
NB, NHG = 4, 4          # batches per core, heads per core
P = 128                 # partitions

DRAIN_QK = int(os.environ.get("K_DRAIN_QK", "3"))
DRAIN_V = int(os.environ.get("K_DRAIN_V", "3"))
DRAIN_S0 = int(os.environ.get("K_DRAIN_S0", "3"))
DRAIN_S1 = int(os.environ.get("K_DRAIN_S1", "3"))


def build_module():
    nc = bacc.Bacc("TRN2", target_bir_lowering=False, debug=False)

    xt = nc.dram_tensor("xt", [NB, P, 4, N], BF16, kind="ExternalInput")
    expbt = nc.dram_tensor("expbt", [P, 64, 512], BF16, kind="ExternalInput")
    wqk = nc.dram_tensor("wqk", [P, 4, 256], BF16, kind="ExternalInput")
    bqk = nc.dram_tensor("bqk", [P, 2], F32, kind="ExternalInput")
    wv = nc.dram_tensor("wv", [P, 4, P], BF16, kind="ExternalInput")
    bv = nc.dram_tensor("bv", [P, 4, 32], F32, kind="ExternalInput")
    wp = nc.dram_tensor("wp", [P, 2, C], BF16, kind="ExternalInput")
    y = nc.dram_tensor("y", [NB, N, C], BF16, kind="ExternalOutput")

    with tile.TileContext(nc) as tc:
        with (
            tc.tile_pool(name="singles", bufs=1) as singles,
            tc.tile_pool(name="xtp", bufs=2) as xtp,
            tc.tile_pool(name="op", bufs=2) as op_pool,
            tc.tile_pool(name="yp", bufs=1) as yp,
            tc.tile_pool(name="rabp", bufs=1) as rabp,
            tc.tile_pool(name="rcbp", bufs=2) as rcbp,
            tc.tile_pool(name="ps_s", bufs=2, space="PSUM") as ps_s,
            tc.tile_pool(name="ps_b", bufs=2, space="PSUM") as ps_b,
        ):
            # ---- resident weights/constants ----
            # first-needed tensors go out on the (otherwise idle) sync
            # queue so the first qk matmul can start ~5us earlier; the
            # big exp(bias) table loads last on the gpsimd queue.
            wqk_sb = singles.tile([P, 4, 256], BF16)
            nc.sync.dma_start(out=wqk_sb, in_=wqk[:])
            bqk_sb = singles.tile([P, 2], F32)
            nc.sync.dma_start(out=bqk_sb, in_=bqk[:])

            xt_tiles = {}

            def load_xt(b):
                xt_t = xtp.tile([P, 4, N], BF16, name="xt_t")
                nc.sync.dma_start(out=xt_t, in_=xt[b])
                xt_tiles[b] = xt_t

            load_xt(0)

            wv_sb = singles.tile([P, 4, P], BF16)
            nc.gpsimd.dma_start(out=wv_sb, in_=wv[:])
            bv_sb = singles.tile([P, 4, 32], F32)
            nc.gpsimd.dma_start(out=bv_sb, in_=bv[:])
            wp_sb = singles.tile([P, 2, C], BF16)
            nc.gpsimd.dma_start(out=wp_sb, in_=wp[:])
            expb_sb = singles.tile([P, 64, 512], BF16)
            for i in range(8):
                nc.gpsimd.dma_start(
                    out=expb_sb[:, 8 * i:8 * i + 8, :], in_=expbt[:, 8 * i:8 * i + 8, :]
                )

            # per-mb softmax-numerator buffers (32 slots each)
            p_sb_a = singles.tile([P, 32, 512], BF16)
            p_sb_b = singles.tile([P, 32, 512], BF16)
            rec_f32 = singles.tile([P, 512], F32)
            # double-buffered q^T/k^T so batch b+1's qk matmuls overlap
            # batch b's score tail
            qkt_a = singles.tile([P, 2, N], BF16)
            qkt_b = singles.tile([P, 2, N], BF16)
            v_sb_a = singles.tile([P, 8, NHG, 64], BF16)
            nc.gpsimd.memset(v_sb_a[:, :, :, 32:33], 1.0)
            nc.gpsimd.memset(v_sb_a[:, :, :, 33:64], 0.0)
            v_sb_b = singles.tile([P, 8, NHG, 64], BF16)
            nc.gpsimd.memset(v_sb_b[:, :, :, 32:33], 1.0)
            nc.gpsimd.memset(v_sb_b[:, :, :, 33:64], 0.0)

            # deferred PE work (PV passes, proj tails) drained into the
            # scores/qk/v streams so the scalar engine never starves
            work_q = []

            def drain(k):
                for _ in range(min(k, len(work_q))):
                    work_q.pop(0)()

            def drain_all():
                drain(len(work_q))

            def queue_pv(mb, o_tiles_, v_sb):
                cur_p = p_sb_a if mb == 0 else p_sb_b
                for pr in range(2):
                    box = {}

                    for nt in range(8):
                        def pv_mms(pr=pr, nt=nt, box=box, mb=mb, cur_p=cur_p):
                            if nt == 0:
                                box["po2"] = ps_b.tile([P, 512], F32, name="po2", tag="b")
                            po2 = box["po2"]
                            for hh in range(2):
                                h = 2 * pr + hh
                                sl = nt * 4 + h
                                nc.tensor.matmul(
                                    po2[64 * hh:64 * hh + 64, :],
                                    lhsT=v_sb[:, nt, h, :],
                                    rhs=cur_p[:, sl, :],
                                    start=(nt == 0),
                                    stop=(nt == 7),
                                    tile_position=(0, 64 * hh),
                                    skip_group_check=True,
                                )
                        work_q.append(pv_mms)

                    def finish(pr=pr, box=box, mb=mb, o_tiles_=o_tiles_):
                        po2 = box["po2"]
                        # 1/denominators (rows 32 / 96): single full-tile
                        # Newton-approx reciprocal straight off PSUM (v's
                        # zero columns keep every po2 row initialized).
                        nc.vector.reciprocal_approx_fast(out=rec_f32[:, :], in_=po2[:, :])
                        # gpsimd partition_broadcast only reads physical
                        # partition 0, so hop each denom row down via a tiny
                        # SBUF->SBUF DMA first. Head A's values fill rows
                        # 0-63 (rows 32-63 multiply po2's zero rows to exact
                        # 0); head B's fill rows 0-95 of a second tile of
                        # which rows 64-95 are consumed (PB can't start at
                        # partition 64).
                        rA = rabp.tile([1, 512], F32, name="rA")
                        rB = rabp.tile([1, 512], F32, name="rB")
                        nc.sync.dma_start(out=rA, in_=rec_f32[32:33, :])
                        nc.sync.dma_start(out=rB, in_=rec_f32[96:97, :])
                        rcb = rcbp.tile([P, 512], F32, name="rcb")
                        nc.gpsimd.partition_broadcast(
                            rcb[0:64, :], rA[0:1, :], channels=64
                        )
                        rcb2 = rcbp.tile([P, 512], F32, name="rcb2")
                        nc.gpsimd.partition_broadcast(
                            rcb2[0:96, :], rB[0:1, :], channels=96
                        )
                        nc.vector.tensor_mul(
                            out=o_tiles_[pr][0:64, mb * 512:(mb + 1) * 512],
                            in0=po2[0:64, :],
                            in1=rcb[0:64, :],
                        )
                        nc.vector.tensor_mul(
                            out=o_tiles_[pr][64:96, mb * 512:(mb + 1) * 512],
                            in0=po2[64:96, :],
                            in1=rcb2[64:96, :],
                        )
                    work_q.append(finish)

            def queue_tail(b_, o_tiles_, box, mts):
                for mt in mts:
                    def proj(mt=mt, o_tiles_=o_tiles_, box=box):
                        if "ysb" not in box:
                            box["ysb"] = yp.tile([P, 8, C], BF16, name="ysb")
                        py = ps_b.tile([P, 512], F32, name="py", tag="b")
                        nc.tensor.matmul(
                            py, lhsT=o_tiles_[0][:, mt * P:(mt + 1) * P],
                            rhs=wp_sb[:, 0, :], start=True, stop=False,
                        )
                        nc.tensor.matmul(
                            py, lhsT=o_tiles_[1][:, mt * P:(mt + 1) * P],
                            rhs=wp_sb[:, 1, :], start=False, stop=True,
                        )
                        nc.vector.tensor_scalar_add(
                            out=box["ysb"][:, mt, :], in0=py, scalar1=0.0
                        )
                    work_q.append(proj)

            def queue_store(b_, box):
                def store(b_=b_, box=box):
                    nc.sync.dma_start(
                        out=y[b_].rearrange("(mt p) c -> p mt c", p=P), in_=box["ysb"]
                    )
                work_q.append(store)

            for b in range(NB):
                xt_sb = xt_tiles.pop(b)
                qkt_sb = qkt_a if b % 2 == 0 else qkt_b

                # ---- q^T / k^T ----
                for mt in range(2):
                    for nb_ in range(2):
                        pq = ps_b.tile([P, 512], F32, name="pq", tag="b")
                        for ko in range(4):
                            nc.tensor.matmul(
                                pq,
                                lhsT=wqk_sb[:, ko, mt * P:(mt + 1) * P],
                                rhs=xt_sb[:, ko, nb_ * 512:(nb_ + 1) * 512],
                                start=(ko == 0),
                                stop=(ko == 3),
                            )
                        nc.vector.tensor_scalar_add(
                            out=qkt_sb[:, mt, nb_ * 512:(nb_ + 1) * 512],
                            in0=pq,
                            scalar1=bqk_sb[:, mt:mt + 1],
                        )
                        drain(DRAIN_QK)
                if b + 1 < NB:
                    load_xt(b + 1)

                # ---- v natural [m, d] ----
                v_sb = v_sb_a if b % 2 == 0 else v_sb_b
                for g in range(2):
                    pv_ = ps_b.tile([P, 4, 4, 32], F32, name="pvv", tag="b")
                    for j in range(4):
                        nt = 4 * g + j
                        for ko in range(4):
                            nc.tensor.matmul(
                                pv_[:, j],
                                lhsT=xt_sb[:, ko, nt * P:(nt + 1) * P],
                                rhs=wv_sb[:, ko, :],
                                start=(ko == 0),
                                stop=(ko == 3),
                            )
                    for j in range(4):
                        nt = 4 * g + j
                        nc.vector.tensor_add(
                            out=v_sb[:, nt, :, 0:32], in0=pv_[:, j], in1=bv_sb
                        )
                    drain(DRAIN_V)

                # ---- per-pair O tiles ----
                o_tiles = []
                for pr in range(2):
                    o_t = op_pool.tile([P, N], BF16, name=f"o{pr}")
                    nc.gpsimd.memset(o_t[96:128, :], 0.0)
                    o_tiles.append(o_t)
                tail_box = {}

                # ---- scores -> exp -> *expb (PV/proj work drained between) ----
                for mb in range(2):
                    cur_p = p_sb_a if mb == 0 else p_sb_b
                    st = None
                    mult_lo = 0
                    for nt in range(8):
                        for h in range(NHG):
                            sl = nt * 4 + h
                            k = sl % 3
                            if k == 0:
                                st = ps_s.tile([P, 3, 512], F32, name="st", tag="s")
                            nc.tensor.matmul(
                                st[:, k, :],
                                lhsT=qkt_sb[32 * h:32 * h + 32, 1, nt * P:(nt + 1) * P],
                                rhs=qkt_sb[32 * h:32 * h + 32, 0, mb * 512:(mb + 1) * 512],
                                tile_position=(32 * h, 0),
                            )
                            if k == 2 or sl == 31:
                                nb_ = k + 1
                                s0 = sl - k
                                nc.scalar.activation(
                                    out=cur_p[:, s0:s0 + nb_, :],
                                    in_=st[:, 0:nb_, :],
                                    func=mybir.ActivationFunctionType.Exp,
                                )
                                hi = s0 + nb_
                                if hi - mult_lo >= 6 or sl == 31:
                                    nc.vector.tensor_mul(
                                        out=cur_p[:, mult_lo:hi, :],
                                        in0=cur_p[:, mult_lo:hi, :],
                                        in1=expb_sb[:, mb * 32 + mult_lo:mb * 32 + hi, :],
                                    )
                                    mult_lo = hi
                        drain(DRAIN_S0 if mb == 0 else DRAIN_S1)
                    queue_pv(mb, o_tiles, v_sb)
                    if mb == 0:
                        queue_tail(b, o_tiles, tail_box, range(0, 4))
                queue_tail(b, o_tiles, tail_box, range(4, 8))
                queue_store(b, tail_box)

            drain_all()

    nc.compile()
    return nc


_NC = None


def _get_nc():
    global _NC
    if _NC is None:
        _NC = build_module()
    return _NC


def _host_prep(x, shared_rel_pos, W_qkv, b_qkv, W_proj, b_proj):
    """Build the 8 per-core input dicts from full inputs."""
    bf16 = ml_dtypes.bfloat16
    x = np.asarray(x, np.float32).reshape(B, N, C)
    W_qkv = np.asarray(W_qkv, np.float32)
    b_qkv = np.asarray(b_qkv, np.float32)
    W_proj = np.asarray(W_proj, np.float32)
    rel = np.asarray(shared_rel_pos, np.float32)

    # xt per batch-group: [4, 128, 4, 1024] with c = ko*128 + p
    xt_groups = []
    for bg in range(2):
        xb = x[NB * bg:NB * (bg + 1)].transpose(0, 2, 1)      # [4, C, N]
        xt = xb.reshape(NB, 4, P, N).transpose(0, 2, 1, 3)    # [4, 128, 4, N]
        xt_groups.append(np.ascontiguousarray(xt).astype(bf16))

    in_maps = []
    for core in range(8):
        bg, hg = core // 4, core % 4
        hs = [hg * NHG + i for i in range(NHG)]

        wqk_f = np.empty((C, 256), np.float32)
        bqk_f = np.empty((P, 2), np.float32)
        wv_f = np.empty((C, P), np.float32)
        bv_f = np.empty((P,), np.float32)
        for i, h in enumerate(hs):
            wqk_f[:, 32 * i:32 * i + 32] = W_qkv[:, 96 * h:96 * h + 32] * SCALE
            wqk_f[:, 128 + 32 * i:128 + 32 * i + 32] = W_qkv[:, 96 * h + 32:96 * h + 64]
            bqk_f[32 * i:32 * i + 32, 0] = b_qkv[96 * h:96 * h + 32] * SCALE
            bqk_f[32 * i:32 * i + 32, 1] = b_qkv[96 * h + 32:96 * h + 64]
            wv_f[:, 32 * i:32 * i + 32] = W_qkv[:, 96 * h + 64:96 * h + 96]
            bv_f[32 * i:32 * i + 32] = b_qkv[96 * h + 64:96 * h + 96]

        wqk_r = wqk_f.reshape(4, P, 256).transpose(1, 0, 2)       # [128, 4, 256]
        wv_r = wv_f.reshape(4, P, P).transpose(1, 0, 2)           # [128, 4, 128]
        bv_r = np.broadcast_to(bv_f.reshape(4, 32), (P, 4, 32))   # [128, 4, 32]

        # pair-split, zero-padded projection weights: [128, 2, 512]
        wp_f = np.zeros((P, 2, C), np.float32)
        for pr in range(2):
            wp_f[0:32, pr] = W_proj[32 * hs[2 * pr]:32 * hs[2 * pr] + 32]
            wp_f[64:96, pr] = W_proj[32 * hs[2 * pr + 1]:32 * hs[2 * pr + 1] + 32]

        # exp(bias)^T in stream order: [128, 64, 512], s = (mb*8+nt)*4+h
        E = np.exp(rel[hs])                                       # [4, n, m]
        expb = E.reshape(NHG, 2, 512, 8, P).transpose(4, 1, 3, 0, 2)
        expb = np.ascontiguousarray(expb).reshape(P, 64, 512).astype(bf16)

        in_maps.append({
            "xt": xt_groups[bg],
            "expbt": expb,
            "wqk": np.ascontiguousarray(wqk_r).astype(bf16),
            "bqk": bqk_f,
            "wv": np.ascontiguousarray(wv_r).astype(bf16),
            "bv": np.ascontiguousarray(bv_r),
            "wp": wp_f.astype(bf16),
        })
    return in_maps


def kernel(x, shared_rel_pos, W_qkv, b_qkv, W_proj, b_proj):
    nc = _get_nc()
    in_maps = _host_prep(x, shared_rel_pos, W_qkv, b_qkv, W_proj, b_proj)
    res = run_bass_kernel_spmd(
        nc, in_maps, core_ids=list(range(8)),
        trace=bool(int(os.environ.get("KERNEL_TRACE", "0"))),
    )
    out = np.zeros((B, N, C), np.float32)
    for core in range(8):
        bg = core // 4
        out[NB * bg:NB * (bg + 1)] += np.asarray(res.results[core]["y"], np.float32)
    out += np.asarray(b_proj, np.float32)
    if res.exec_time_ns is not None:
        kernel.last_exec_time_ns = res.exec_time_ns
    return out.reshape(B, 32, 32, C)


kernel.last_exec_time_ns = None
